# revision 48
# baseline (speedup 1.0000x reference)
"""Single-head causal attention (B=8, T=4096, EMB=1024, HEAD=64) on 8 trn2 cores.

Strategy: data-parallel over batch, one batch element per NeuronCore.

t-tile-streamed pipeline (8 tiles of 512 query positions) designed around
three measured hardware realities: LDWEIGHTS serializes with matmuls (so
weight reloads must be rare or hidden via row-tiled pairs), the ACT engine's
exp stream is a ~75us fixed cost that must start early and never starve, and
the PE clock halves (HAM) whenever the tensor engine idles >3.4us.

Per core (all matmuls bf16, fp32 PSUM):
  - Projection in tile-PAIRS with 1024-wide moving operands (one weight load
    per 8 matmuls): pass A stationary [Wk|Wq] -> [K^T; Q^T], pass B
    stationary [Wv|Wk] -> [V^T; K^T]. K^T/Q^T land on both partition halves
    (one gpsimd-DMA dup for Q^T-lo) so score matmuls can run as concurrent
    row-tiled lo/hi pairs (contraction is only d=64). V natural chunks via
    PE transposes (is_transpose at base partition 0 only; XBAR dma-transpose
    and col tiling are broken on hw).
  - Scores S^T chunk [128s, 512t] per (s-chunk, tile); exp on ACT from PSUM
    in 2-chunk [128,1024] groups, bf16 out; sub-diagonal garbage zeroed on
    GpSimd, diagonal block masked on DVE.
  - PV transposed: out^T[65,512] += [V_aa|ones]^T @ P^T[aa] — 65-column
    stationary keeps the weight reload small; ones column accumulates the
    softmax denominator. PV lags the exp stream by two groups.
  - out^T tiles -> DRAM [65, T]; host divides by the Z row and transposes.

A warmup matmul burst (zeroed tile) runs during the first x DMA so the HAM
clock gate is already at 2.4GHz when real work arrives.
"""

import os

import numpy as np
import ml_dtypes

B, T, EMB, HEAD = 8, 4096, 1024, 64
KCH = EMB // 128           # 8 contraction chunks
NTT = T // 512             # 8 t-tiles
NTS = T // 128             # 32 s-chunks
BF16 = ml_dtypes.bfloat16

# NOTE: col tiling (tile_position=(0, 64)) crashes trn2 hw — never use it.
# NOTE: is_transpose at base partition 64 also crashes hw.
# NOTE: dma transpose=True (XBAR) returns wrong data on hw (sim-only OK).
SC_PAIR = os.environ.get("BASS_SC_PAIR", "1") != "0"    # row-tiled score pairs

_CACHE = {}


def _build_program():
    import concourse.bacc as bacc
    import concourse.tile as tile
    from concourse import mybir
    from concourse.masks import make_identity

    if os.environ.get("BASS_LDWOPT", "1") != "0":
        # the env default pins --enable-ldw-opt=false; weight-load
        # pipelining is worth real time here
        from concourse.compiler_utils import get_compiler_flags, set_compiler_flags

        set_compiler_flags(
            [
                f.replace("--enable-ldw-opt=false", "--enable-ldw-opt=true")
                for f in get_compiler_flags()
            ]
        )

    fp32 = mybir.dt.float32
    bf16 = mybir.dt.bfloat16
    EXP = mybir.ActivationFunctionType.Exp

    nc = bacc.Bacc("TRN2", target_bir_lowering=False, debug=False)
    xj_ap = nc.dram_tensor("xj", [NTT, 128, KCH, 512], bf16, kind="ExternalInput").ap()
    # per k-chunk 256 cols: [Wk | Wq] then [Wv | Wk]
    w_ap = nc.dram_tensor("w", [128, KCH * 256], bf16, kind="ExternalInput").ap()
    mask_ap = nc.dram_tensor("mask", [128, 128], bf16, kind="ExternalInput").ap()
    o_ap = nc.dram_tensor("o", [65, T], fp32, kind="ExternalOutput").ap()

    with tile.TileContext(nc) as tc:
        with (
            tc.tile_pool(name="consts", bufs=1) as consts,
            tc.tile_pool(name="ps_scr", bufs=1, space="PSUM") as ps_scr,
            tc.tile_pool(name="ps_sc", bufs=2, space="PSUM") as ps_sc,
            tc.tile_pool(name="ps_pv", bufs=1, space="PSUM") as ps_pv,
        ):
            # ---------- constants / prologue ----------
            # warm the PE's HAM clock gate with throwaway matmuls on a
            # zeroed tile while the first x tile is still in flight
            warm_sb = consts.tile([128, 512], bf16, tag="warm")
            nc.gpsimd.memset(warm_sb, 0.0)
            scrw = ps_scr.tile([128, 512], fp32, tag="scr")
            for r in range(8):
                nc.tensor.matmul(
                    scrw,
                    warm_sb[:, 0:128],
                    warm_sb,
                    start=(r == 0),
                    stop=(r == 7),
                    skip_group_check=True,
                )

            w_sb = consts.tile([128, KCH * 256], bf16, tag="w")
            nc.sync.dma_start(out=w_sb, in_=w_ap)
            mask_sb = consts.tile([128, 128], bf16, tag="mask")
            nc.sync.dma_start(out=mask_sb, in_=mask_ap)

            xt_sb = consts.tile([128, NTT, KCH, 512], bf16, tag="xt")
            # tiles 0/1 split per k-chunk so the first projections start as
            # soon as their chunks land; later tiles alternate between the
            # two hardware DMA queues (SP / ACT) to halve queue serialization
            for j in range(2):
                for k in range(KCH):
                    nc.sync.dma_start(out=xt_sb[:, j, k], in_=xj_ap[j, :, k])
            for j in range(2, NTT):
                eng = nc.sync if j % 2 == 0 else nc.scalar
                eng.dma_start(out=xt_sb[:, j], in_=xj_ap[j])

            ident_sb = consts.tile([64, 64], fp32, tag="ident")
            make_identity(nc, ident_sb)

            # V natural chunks [128s, 64d | ones] per s-chunk
            vt_sb = consts.tile([128, NTS * 65], bf16, tag="vt")
            nc.gpsimd.memset(vt_sb, 1.0)

            ks_sb = consts.tile([128, T], bf16, tag="ks")   # K^T on both halves
            qs_sb = consts.tile([128, T], bf16, tag="qs")   # Q^T on both halves
            vts_sb = consts.tile([64, NTT * 512], fp32, tag="vts")  # V^T staging
            pt_sb = consts.tile([128, NTS * 512], bf16, tag="pt")   # P^T chunks
            ot_sb = consts.tile([65, T], fp32, tag="ot")            # out^T

            # ---------- projection emitters (one tile) ----------
            def emit_proj_kq(j):
                lo, hi = j * 512, (j + 1) * 512
                scr = ps_scr.tile([128, 512], fp32, tag="scr")
                for k in range(KCH):
                    nc.tensor.matmul(
                        scr,
                        w_sb[:, k * 256:k * 256 + 128],
                        xt_sb[:, j, k],
                        start=(k == 0),
                        stop=(k == KCH - 1),
                        skip_group_check=True,
                    )
                nc.vector.tensor_copy(ks_sb[0:64, lo:hi], scr[0:64, :])
                nc.vector.tensor_copy(qs_sb[64:128, lo:hi], scr[64:128, :])
                # Q^T -> low partitions via gpsimd software DGE (own queue)
                nc.gpsimd.dma_start(out=qs_sb[0:64, lo:hi], in_=qs_sb[64:128, lo:hi])
                return scr

            def emit_proj_v(j, scr):
                lo, hi = j * 512, (j + 1) * 512
                for k in range(KCH):
                    nc.tensor.matmul(
                        scr,
                        w_sb[:, k * 256 + 128:k * 256 + 256],
                        xt_sb[:, j, k],
                        start=(k == 0),
                        stop=(k == KCH - 1),
                        skip_group_check=True,
                    )
                nc.vector.tensor_copy(vts_sb[:, lo:hi], scr[0:64, :])
                nc.vector.tensor_copy(ks_sb[64:128, lo:hi], scr[64:128, :])
                # V natural via PE transposes (base partition 0 only)
                for c in range(4):
                    i = 4 * j + c
                    nc.tensor.matmul(
                        scr[:, 256 + c * 64:256 + c * 64 + 64],
                        vts_sb[0:64, i * 128:(i + 1) * 128],
                        ident_sb,
                        is_transpose=True,
                        start=(c == 0),
                        stop=(c == 3),
                        skip_group_check=True,
                    )
                for c in range(4):
                    i = 4 * j + c
                    nc.vector.tensor_copy(
                        vt_sb[:, i * 65:i * 65 + 64],
                        scr[:, 256 + c * 64:256 + c * 64 + 64],
                    )

            # ---------- score / exp / PV emitters per tile ----------
            def make_scpv(j):
                jsl = slice(j * 512, (j + 1) * 512)
                po = ps_pv.tile([65, 512], fp32, tag="pv")
                nchunk = 4 * j + 4
                groups = [list(range(g, min(g + 3, nchunk))) for g in range(0, nchunk, 3)]
                ngrp = len(groups)

                def emit_sc(gi, j=j, jsl=jsl, groups=groups):
                    chunks = groups[gi]
                    sc = ps_sc.tile([128, 1536], fp32, tag="sc")
                    for ci, a in enumerate(chunks):
                        half = 64 if (SC_PAIR and a % 2 == 1) else 0
                        nc.tensor.matmul(
                            sc[:, ci * 512:(ci + 1) * 512],
                            ks_sb[half:half + 64, a * 128:(a + 1) * 128],
                            qs_sb[half:half + 64, jsl],
                            start=True,
                            stop=True,
                            skip_group_check=True,
                        )
                    cnt = len(chunks)
                    a0 = chunks[0]
                    nc.scalar.activation(
                        pt_sb[:, a0 * 512:(a0 + cnt) * 512],
                        sc[:, 0:cnt * 512],
                        EXP,
                        scale=0.125,
                    )
                    for a in chunks:
                        if a >= 4 * j:
                            sub = a - 4 * j
                            if sub > 0:
                                nc.gpsimd.memset(
                                    pt_sb[:, a * 512:a * 512 + 128 * sub], 0.0
                                )
                            dsl = slice(a * 512 + 128 * sub, a * 512 + 128 * sub + 128)
                            nc.vector.tensor_mul(pt_sb[:, dsl], pt_sb[:, dsl], mask_sb)

                def emit_pv(gi, po=po, nchunk=nchunk, groups=groups):
                    for aa in groups[gi]:
                        nc.tensor.matmul(
                            po,
                            vt_sb[:, aa * 65:(aa + 1) * 65],
                            pt_sb[:, aa * 512:(aa + 1) * 512],
                            start=(aa == 0),
                            stop=(aa == nchunk - 1),
                            skip_group_check=True,
                        )

                def emit_tail(jsl=jsl, po=po, ngrp=ngrp, emit_pv=emit_pv):
                    emit_pv(ngrp - 1)
                    nc.vector.tensor_copy(ot_sb[:, jsl], po)
                    # output store on the gpsimd software-DGE queue (keeps
                    # the SP queue free for x tiles)
                    nc.gpsimd.dma_start(out=o_ap[:, jsl], in_=ot_sb[:, jsl])

                return emit_sc, emit_pv, emit_tail, ngrp

            # ---------- flat software pipeline ----------
            # proj for tiles [..] is emitted DURING the keyed tile's group
            # loop so the exp stream never waits on projection at a tile
            # boundary
            prev_tail = [None]
            emit_proj_v(0, emit_proj_kq(0))

            for j in range(NTT):
                emit_sc, emit_pv, emit_tail, ngrp = make_scpv(j)
                inserts = {}
                if j + 1 < NTT:
                    holder = {}

                    def ins_kq(jn=j + 1, holder=holder):
                        holder["scr"] = emit_proj_kq(jn)

                    def ins_vn(jn=j + 1, holder=holder):
                        emit_proj_v(jn, holder["scr"])

                    kqs = min(1, ngrp - 1)
                    vns = min(3, ngrp - 1)
                    inserts.setdefault(kqs, []).append(ins_kq)
                    inserts.setdefault(vns, []).append(ins_vn)

                lag = 1 if j == NTT - 1 else 2
                # previous tile's drain: as late as possible (so ACT gets a
                # two-group backlog across the boundary) but before this
                # tile's exp overwrites the P^T chunks its last PV group
                # still reads (deps are tracked in emission order)
                prev_ngrp = (4 * j + 2) // 3  # ceil(4j/3) = groups of tile j-1
                tail_slot = max(0, min(1, prev_ngrp - 2, ngrp - 1))
                for g in range(ngrp):
                    emit_sc(g)
                    if g == tail_slot and prev_tail[0] is not None:
                        prev_tail[0]()
                        prev_tail[0] = None
                    for fn in inserts.get(g, []):
                        fn()
                    if g >= lag:
                        emit_pv(g - lag)
                if lag == 2 and ngrp >= 2:
                    emit_pv(ngrp - 2)
                prev_tail[0] = emit_tail

            prev_tail[0]()

    nc.compile()
    return nc


def _get_nc():
    if "nc" not in _CACHE:
        _CACHE["nc"] = _build_program()
    return _CACHE["nc"]


def _make_in_maps(x, W):
    x = np.asarray(x, dtype=np.float32)
    W = np.asarray(W, dtype=np.float32)
    assert x.shape == (B, T, EMB) and W.shape == (EMB, 3 * HEAD)

    wc = W.astype(BF16).reshape(KCH, 128, 192)  # [k, p, c]: [Wk | Wq | Wv]
    wj = (
        np.concatenate([wc, wc[:, :, 0:64]], axis=2)  # append Wk again
        .transpose(1, 0, 2)
        .reshape(128, KCH * 256)
        .copy()
    )
    mask = np.triu(np.ones((128, 128), np.float32)).astype(BF16)
    in_maps = []
    for b in range(B):
        xb = x[b].astype(BF16)  # [T, EMB]
        # xj[j, p, k, c] = x[512j + c, 128k + p]
        xj = np.ascontiguousarray(
            xb.reshape(NTT, 512, KCH, 128).transpose(0, 3, 2, 1)
        )
        in_maps.append({"xj": xj, "w": wj, "mask": mask})
    return in_maps


def _postprocess(o):
    # o: [65, T] fp32 -> [T, HEAD] normalized
    return (o[0:HEAD, :] / o[HEAD:HEAD + 1, :]).T


def kernel(x, W):
    from concourse.bass_utils import run_bass_kernel_spmd

    nc = _get_nc()
    in_maps = _make_in_maps(x, W)
    res = run_bass_kernel_spmd(nc, in_maps, list(range(B)))
    return np.stack(
        [_postprocess(res.results[b]["o"]) for b in range(B)]
    ).astype(np.float32)
